# revision 29
# baseline (speedup 1.0000x reference)
"""Causal self-attention (B=4, T=2048, C=1024, H=16) on 8 TRN2 NeuronCores.

Strategy: tensor-parallel over heads. Each core computes QKV for its 2 heads
(columns of W_attn), runs causal flash-style attention for those heads over
all batches, then AllToAll collectives redistribute Y^T (head-dim-major) so
each core holds all 1024 head-dims for its 1/8 token slice. Each core then
computes its disjoint token slice of the output projection; the host
concatenates the 8 slices.

Layouts (per core):
  xT      [C, B*T]   bf16  x transposed (host-staged)
  wqk     [C, 256]   bf16  W_attn cols [qA|qB|kA|kB] for the core's 2 heads
  wv      [C, 128]   bf16  W_attn v-cols [vA|vB]
  bqk     [256, 1]   f32   matching biases
  bv      [128, 1]   f32
  wpj     [C, C]     bf16  W_proj (replicated)
  bpj     [1, C]     f32   b_proj (replicated)

Attention per (batch, head-pair): scores are computed transposed,
S^T[key, q] = K^T.T @ Q^T, both heads row-packed into the 128x128 PE array
(K=64 each) writing one 2-bank PSUM tile (one exp covers both heads).
exp on ScalarE (scale=1/8 fused, no max-subtraction: scores are O(1));
causal handled by skipping k-tiles above the diagonal, slicing the exp to
the valid q-range, and one [128,128] triangular mask multiply per head per
diagonal k-tile.  AV uses V' = [ones64 | V] as the stationary operand so
the softmax denominator lands replicated on PSUM rows 0-63 (y on rows
64-127); normalization is one fast approximate reciprocal plus a multiply.

Scheduling: attention runs in two passes (even q-chunks of every batch,
then odd q-chunks).  The first AllToAll covers the even-chunk halves and
overlaps the second pass; the first half of the projection is interleaved
into the last batch's second pass.  The next batch's QKV work is emitted
as a queue of small bursts injected between attention k-tiles to keep the
PE dense (HAM warm) through the ACT-bound attention phases.
"""

import numpy as np
import ml_dtypes

import concourse.bass as bass
import concourse.bacc as bacc
import concourse.mybir as mybir
import concourse.tile as tile
from concourse.tile import add_dep_helper
from concourse.bass_utils import run_bass_kernel_spmd

N_CORES = 8
B, T, C, H, D = 4, 2048, 1024, 16, 64

BF16 = mybir.dt.bfloat16
F32 = mybir.dt.float32


def _build(n_cores=N_CORES, b=B, t=T, c=C):
    """Build the SPMD graph (identical on every core)."""
    nt = b * t                 # total tokens
    s = nt // n_cores          # tokens per output slice
    hs = s // 2                # half-slice (per-A2A) width
    ch = min(512, t)           # attention q-chunk width
    n_ch = t // ch             # chunks per batch
    nkc = c // 128             # contraction tiles over C
    n_vt = t // 128            # V token-tiles per batch
    n_tt = s // 128            # proj token-tiles per core

    nc = bacc.Bacc("TRN2", target_bir_lowering=False, debug=False,
                   num_devices=n_cores)

    xT = nc.dram_tensor("xT", [c, nt], BF16, kind="ExternalInput")
    wqk = nc.dram_tensor("wqk", [c, 256], BF16, kind="ExternalInput")
    wv = nc.dram_tensor("wv", [c, 128], BF16, kind="ExternalInput")
    bqk = nc.dram_tensor("bqk", [256, 1], F32, kind="ExternalInput")
    bv = nc.dram_tensor("bv", [128, 1], F32, kind="ExternalInput")
    wpj = nc.dram_tensor("wpj", [c, c], BF16, kind="ExternalInput")
    bpj = nc.dram_tensor("bpj", [1, c], F32, kind="ExternalInput")
    out_ext = nc.dram_tensor("out", [s, c], F32, kind="ExternalOutput")

    with tile.TileContext(nc) as tc:
        with (
            tc.tile_pool(name="sb", bufs=1) as sb,
            tc.tile_pool(name="ps", bufs=1, space="PSUM") as ps,
            tc.tile_pool(name="dr", bufs=1, space="DRAM") as dr,
        ):
            y_loc = [dr.tile([c, hs], BF16, name=f"y_loc{h}",
                             tag=f"y_loc{h}") for h in range(2)]
            y_gth = [dr.tile([c, hs], BF16, name=f"y_gth{h}",
                             tag=f"y_gth{h}") for h in range(2)]

            # ---- constants (no DMA deps, so compute starts early) ----
            trimask = sb.tile([128, 128], BF16, name="trimask", tag="trimask")
            nc.gpsimd.memset(trimask[:], 0.0)
            nc.gpsimd.affine_select(   # trimask[r, q] = 1 iff q >= r
                out=trimask[:], in_=trimask[:],
                compare_op=mybir.AluOpType.is_gt, fill=1.0,
                base=0, pattern=[[-1, 128]], channel_multiplier=1,
            )
            ident = sb.tile([128, 128], BF16, name="ident", tag="ident")
            nc.gpsimd.memset(ident[:], 0.0)
            nc.gpsimd.affine_select(
                out=ident[:], in_=ident[:],
                compare_op=mybir.AluOpType.not_equal, fill=1.0,
                base=0, pattern=[[-1, 128]], channel_multiplier=1,
            )
            ones64 = sb.tile([65, 128], F32, name="ones64", tag="ones64")
            nc.vector.memset(ones64[64:65, :], 1.0)

            bq_sb = sb.tile([128, 1], F32, name="bq_sb", tag="bq_sb")
            bk_sb = sb.tile([128, 1], F32, name="bk_sb", tag="bk_sb")
            bv_sb = sb.tile([128, 1], F32, name="bv_sb", tag="bv_sb")
            nc.sync.dma_start(bq_sb[:], bqk.ap()[0:128, :])
            nc.sync.dma_start(bk_sb[:], bqk.ap()[128:256, :])

            # batch-0 x^T tiles first in the DMA queue (critical path), with
            # the small attention weights interleaved just behind xt0
            xt0 = []
            wqk_sb, wv_sb = [], []
            for kc in range(nkc):
                x_t = sb.tile([128, t], BF16, name=f"xt0_{kc}",
                              tag="xt", bufs=nkc + 4)
                eng = nc.sync if kc % 2 == 0 else nc.scalar
                eng.dma_start(x_t[:], xT.ap()[128 * kc:128 * (kc + 1), 0:t])
                xt0.append(x_t)
                wq_t = sb.tile([128, 256], BF16, name=f"wqk{kc}", tag="wqk",
                               bufs=nkc)
                nc.sync.dma_start(wq_t[:], wqk.ap()[128 * kc:128 * (kc + 1), :])
                wqk_sb.append(wq_t)
            nc.scalar.dma_start(bv_sb[:], bv.ap())
            for kc in range(nkc):
                wv_t = sb.tile([128, 128], BF16, name=f"wv{kc}", tag="wv",
                               bufs=nkc)
                nc.scalar.dma_start(wv_t[:],
                                    wv.ap()[128 * kc:128 * (kc + 1), :])
                wv_sb.append(wv_t)

            def qkv_work(bi, st, xt_pre=None):
                """Generator emitting batch bi's x^T load + Q^T/K^T/V' compute
                in small bursts (injected between attention k-tiles)."""
                if xt_pre is not None:
                    xt = xt_pre
                else:
                    xt = []
                    for kc in range(nkc):
                        x_t = sb.tile([128, t], BF16, name=f"xt{bi}_{kc}",
                                      tag="xt", bufs=nkc + 4)
                        nc.sync.dma_start(
                            x_t[:], xT.ap()[128 * kc:128 * (kc + 1),
                                            t * bi:t * (bi + 1)])
                        xt.append(x_t)
                        if kc % 4 == 3:
                            yield
                qt_sb = sb.tile([128, t], BF16, name=f"qt{bi}", tag="qt",
                                bufs=b)
                kt_sb = sb.tile([128, t], BF16, name=f"kt{bi}", tag="kt",
                                bufs=b)
                st["qt"], st["kt"] = qt_sb, kt_sb
                for half, (dst, bias) in enumerate(((qt_sb, bq_sb),
                                                    (kt_sb, bk_sb))):
                    for n0 in range(0, t, 512):
                        pqk = ps.tile([128, 512], F32,
                                      name=f"pqk{bi}_{half}_{n0}",
                                      tag="qkps", bufs=2)
                        for kc in range(nkc):
                            nc.tensor.matmul(
                                pqk[:],
                                wqk_sb[kc][:, 128 * half:128 * (half + 1)],
                                xt[kc][:, n0:n0 + 512],
                                start=(kc == 0), stop=(kc == nkc - 1))
                            if kc == nkc // 2 - 1:
                                yield
                        nc.vector.tensor_scalar_add(dst[:, n0:n0 + 512],
                                                    pqk[:], bias[:])
                        yield
                # V^T (D-major, per-partition bias) then PE-transpose into
                # token-major V' tiles [128, 256] = [ones|vA|ones|vB]
                vt_sb = sb.tile([128, t], BF16, name=f"vt{bi}", tag="vt",
                                bufs=2)
                for n0 in range(0, t, 512):
                    pvt = ps.tile([128, 512], F32, name=f"pvt{bi}_{n0}",
                                  tag="qkps", bufs=2)
                    for kc in range(nkc):
                        nc.tensor.matmul(pvt[:], wv_sb[kc][:],
                                         xt[kc][:, n0:n0 + 512],
                                         start=(kc == 0), stop=(kc == nkc - 1))
                        if kc == nkc // 2 - 1:
                            yield
                    nc.vector.tensor_scalar_add(vt_sb[:, n0:n0 + 512],
                                                pvt[:], bv_sb[:])
                    yield
                v_sb = st["v"] = []
                for tt in range(n_vt):
                    ptr = ps.tile([128, 128], BF16, name=f"ptr{bi}_{tt}",
                                  tag="qkps", bufs=2)
                    nc.tensor.transpose(ptr[:],
                                        vt_sb[:, 128 * tt:128 * (tt + 1)],
                                        ident[:])
                    v_t = sb.tile([128, 256], BF16, name=f"v{bi}_{tt}",
                                  tag="v", bufs=b * n_vt)
                    nc.vector.memset(v_t[:, 0:64], 1.0)
                    nc.vector.memset(v_t[:, 128:192], 1.0)
                    nc.vector.tensor_copy(v_t[:, 64:128], ptr[:, 0:64])
                    nc.vector.tensor_copy(v_t[:, 192:256], ptr[:, 64:128])
                    v_sb.append(v_t)
                    if tt % 2 == 1:
                        yield

            fence = [None]   # last attention-side instruction (ordering)

            def attn_chunk(bi, ci, st, inject, ys_eng=None):
                """Emit attention for (batch bi, q-chunk ci)."""
                if ys_eng is None:
                    ys_eng = nc.gpsimd
                qt_sb, kt_sb, v_sb = st["qt"], st["kt"], st["v"]
                q0 = ch * ci
                n_k = (q0 + ch) // 128
                ya = ps.tile([128, ch], F32, name=f"ya{bi}_{ci}",
                             tag="y", bufs=2)
                yb = ps.tile([128, ch], F32, name=f"yb{bi}_{ci}",
                             tag="y", bufs=2)
                # software pipeline: AV lags scores/exp by LAG k-tiles so
                # the PE never waits on ScalarE's exp (keeps PE dense/warm)
                LAG = 2
                p_tiles = {}
                for step in range(n_k + LAG):
                    if step < n_k:
                        kt = step
                        j0 = max(0, 128 * kt - q0)
                        # both heads' scores in one 2-bank psum tile:
                        # cols [0,ch) = head A, [ch,2ch) = head B
                        sAB = ps.tile([128, 2 * ch], F32,
                                      name=f"sAB{bi}_{ci}_{kt}",
                                      tag="sc", bufs=2)
                        nc.tensor.matmul(
                            sAB[:, j0:ch],
                            kt_sb[0:64, 128 * kt:128 * (kt + 1)],
                            qt_sb[0:64, q0 + j0:q0 + ch],
                            start=True, stop=True, tile_position=(0, 0))
                        nc.tensor.matmul(
                            sAB[:, ch + j0:2 * ch],
                            kt_sb[64:128, 128 * kt:128 * (kt + 1)],
                            qt_sb[64:128, q0 + j0:q0 + ch],
                            start=True, stop=True, tile_position=(64, 0))
                        pAB = sb.tile([128, 2 * ch], BF16,
                                      name=f"pAB{bi}_{ci}_{kt}",
                                      tag="p", bufs=6)
                        nc.scalar.activation(pAB[:, j0:2 * ch],
                                             sAB[:, j0:2 * ch],
                                             mybir.ActivationFunctionType.Exp,
                                             scale=0.125)
                        if 128 * kt >= q0:   # diagonal: triangular mask
                            nc.vector.tensor_mul(pAB[:, j0:j0 + 128],
                                                 pAB[:, j0:j0 + 128],
                                                 trimask[:])
                            nc.vector.tensor_mul(
                                pAB[:, ch + j0:ch + j0 + 128],
                                pAB[:, ch + j0:ch + j0 + 128], trimask[:])
                        p_tiles[kt] = pAB
                    if step >= LAG:
                        kt = step - LAG
                        j0 = max(0, 128 * kt - q0)
                        pAB = p_tiles.pop(kt)
                        nc.tensor.matmul(ya[:, j0:ch], v_sb[kt][:, 0:128],
                                         pAB[:, j0:ch],
                                         start=(kt == 0),
                                         stop=(kt == n_k - 1),
                                         skip_group_check=True)
                        nc.tensor.matmul(yb[:, j0:ch], v_sb[kt][:, 128:256],
                                         pAB[:, ch + j0:2 * ch],
                                         start=(kt == 0),
                                         stop=(kt == n_k - 1),
                                         skip_group_check=True)
                    inject()
                # normalize: den replicated on psum rows 0-63, y on 64-127
                rsA = sb.tile([64, ch], F32, name=f"rsA{bi}_{ci}",
                              tag="rs", bufs=2)
                rsB = sb.tile([64, ch], F32, name=f"rsB{bi}_{ci}",
                              tag="rs", bufs=2)
                nc.vector.reciprocal_approx_fast(rsA[:], ya[0:64, :])
                nc.vector.reciprocal_approx_fast(rsB[:], yb[0:64, :])
                ysA = sb.tile([64, ch], BF16, name=f"ysA{bi}_{ci}",
                              tag="ys", bufs=4)
                ysB = sb.tile([64, ch], BF16, name=f"ysB{bi}_{ci}",
                              tag="ys", bufs=4)
                nc.vector.tensor_mul(ysA[:], ya[64:128, :], rsA[:])
                nc.vector.tensor_mul(ysB[:], yb[64:128, :], rsB[:])
                # store into A2A-input halves: shard j rows, half h
                u0 = 0
                while u0 < ch:
                    g = bi * t + q0 + u0          # global token
                    j = g // s
                    col = g % s
                    h = 0 if col < hs else 1
                    col_h = col - h * hs
                    seg = min(ch - u0, hs - col_h)
                    ys_eng.dma_start(
                        y_loc[h][128 * j:128 * j + 64, col_h:col_h + seg],
                        ysA[:, u0:u0 + seg])
                    fence[0] = ys_eng.dma_start(
                        y_loc[h][128 * j + 64:128 * (j + 1), col_h:col_h + seg],
                        ysB[:, u0:u0 + seg])
                    u0 += seg
                inject()

            def after_attn(inst):
                if fence[0] is not None:
                    add_dep_helper(inst.ins, fence[0].ins, sync=False,
                                   reason="post-attn ordering")
                return inst

            def proj_work(h, yg_pre=None):
                """Emit the output projection for half h of my token slice."""
                if yg_pre is not None:
                    yg_sb = yg_pre
                else:
                    yg_sb = []
                    engs = (nc.gpsimd, nc.sync, nc.scalar)
                    for kc in range(nkc):
                        yg_t = sb.tile([128, hs], BF16, name=f"yg{h}_{kc}",
                                       tag="yg", bufs=2 * nkc)
                        after_attn(engs[kc % 3].dma_start(
                            yg_t[:], y_gth[h][128 * kc:128 * (kc + 1), :]))
                        yg_sb.append(yg_t)
                for tt in range(n_tt // 2):
                    o_sb = sb.tile([128, c], F32, name=f"os{h}_{tt}",
                                   tag="os", bufs=2)
                    for n0 in range(0, c, 512):
                        po = ps.tile([128, 512], F32, name=f"po{h}_{tt}_{n0}",
                                     tag="qkps", bufs=2)
                        for kc in range(nkc):
                            after_attn(nc.tensor.matmul(
                                po[:],
                                yg_sb[kc][:, 128 * tt:128 * (tt + 1)],
                                wpj_sb[kc][:, n0:n0 + 512],
                                start=(kc == 0), stop=(kc == nkc - 1)))
                        nc.vector.tensor_add(o_sb[:, n0:n0 + 512], po[:],
                                             bpj_bc[:, n0:n0 + 512])
                    oeng = nc.sync if tt % 2 == 0 else nc.scalar
                    oeng.dma_start(
                        out_ext.ap()[hs * h + 128 * tt:
                                     hs * h + 128 * (tt + 1), :], o_sb[:])

            # ---- batch 0 QKV inline ----
            states = [{}]
            for _ in qkv_work(0, states[0], xt_pre=xt0):
                pass

            # proj weights + bias broadcast (DMA overlaps batch-0 attention)
            wpj_sb = []
            for kc in range(nkc):
                wp_t = sb.tile([128, c], BF16, name=f"wpj{kc}", tag="wpj",
                               bufs=nkc)
                nc.sync.dma_start(wp_t[:], wpj.ap()[128 * kc:128 * (kc + 1), :])
                wpj_sb.append(wp_t)
            brow = sb.tile([65, c], F32, name="brow", tag="brow")
            nc.sync.dma_start(brow[64:65, 0:c], bpj.ap())
            bpj_bc = sb.tile([128, c], F32, name="bpj_bc", tag="bpj_bc")
            for n0 in range(0, c, 512):
                bb_ps = ps.tile([128, 512], F32, name=f"bb_ps{n0}",
                                tag="qkps", bufs=2)
                nc.tensor.matmul(bb_ps[:],
                                 ones64[64:65, 0:128], brow[64:65, n0:n0 + 512],
                                 start=True, stop=True)
                nc.vector.tensor_copy(bpj_bc[:, n0:n0 + 512], bb_ps[:])

            # ---- pass 1: even chunks, next batch's QKV interleaved ----
            if n_ch >= 2:
                passes = ([ci for ci in range(n_ch) if ci % 2 == 0],
                          [ci for ci in range(n_ch) if ci % 2 == 1])
            else:
                passes = ([0], [])
            pending = iter(())
            for bi in range(b):
                for _ in pending:
                    pass
                if bi + 1 < b:
                    states.append({})
                    pending = qkv_work(bi + 1, states[bi + 1])
                else:
                    pending = iter(())

                def inject2():
                    next(pending, None)
                    next(pending, None)
                for ci in passes[0]:
                    attn_chunk(bi, ci, states[bi], inject2)
            for _ in pending:
                pass

            # A2A #1: even-chunk halves (overlaps pass-2 compute)
            nc.gpsimd.collective_compute(
                "AllToAll", mybir.AluOpType.bypass,
                replica_groups=[list(range(n_cores))],
                ins=[y_loc[0].opt()], outs=[y_gth[0].opt()],
            )

            # ---- pass 2: odd chunks ----
            def noop():
                pass
            for bi in range(b):
                for ci in passes[1]:
                    attn_chunk(bi, ci, states[bi], noop)

            # gathered half-0 loads, A2A #2, then both projection halves
            nc.gpsimd.collective_compute(
                "AllToAll", mybir.AluOpType.bypass,
                replica_groups=[list(range(n_cores))],
                ins=[y_loc[1].opt()], outs=[y_gth[1].opt()],
            )
            yg0 = []
            engs0 = (nc.gpsimd, nc.sync, nc.scalar)
            for kc in range(nkc):
                yg_t = sb.tile([128, hs], BF16, name=f"yg0_{kc}",
                               tag="yg", bufs=2 * nkc)
                after_attn(engs0[kc % 3].dma_start(
                    yg_t[:], y_gth[0][128 * kc:128 * (kc + 1), :]))
                yg0.append(yg_t)
            proj_work(0, yg0)
            proj_work(1)

    nc.compile()
    return nc


def _in_maps(x, W_attn, b_attn, W_proj, b_proj, n_cores=N_CORES):
    bsz, t, c = x.shape
    xT = np.ascontiguousarray(
        x.reshape(bsz * t, c).T).astype(ml_dtypes.bfloat16)
    wpj = W_proj.astype(ml_dtypes.bfloat16)
    bpj = b_proj.reshape(1, c).astype(np.float32)
    maps = []
    hpc = 2
    d = c // 16
    for i in range(n_cores):
        cols, bcols, vcols, bvcols = [], [], [], []
        for h in (hpc * i, hpc * i + 1):
            cols.append(W_attn[:, d * h:d * (h + 1)])          # q
            bcols.append(b_attn[d * h:d * (h + 1)])
        for h in (hpc * i, hpc * i + 1):
            cols.append(W_attn[:, c + d * h:c + d * (h + 1)])  # k
            bcols.append(b_attn[c + d * h:c + d * (h + 1)])
        for h in (hpc * i, hpc * i + 1):
            vcols.append(W_attn[:, 2 * c + d * h:2 * c + d * (h + 1)])
            bvcols.append(b_attn[2 * c + d * h:2 * c + d * (h + 1)])
        maps.append({
            "xT": xT,
            "wqk": np.concatenate(cols, axis=1).astype(ml_dtypes.bfloat16),
            "wv": np.concatenate(vcols, axis=1).astype(ml_dtypes.bfloat16),
            "bqk": np.concatenate(bcols).reshape(256, 1).astype(np.float32),
            "bv": np.concatenate(bvcols).reshape(128, 1).astype(np.float32),
            "wpj": wpj,
            "bpj": bpj,
        })
    return maps


_NC_CACHE = {}


def kernel(x, W_attn, b_attn, W_proj, b_proj, _trace=False):
    x = np.asarray(x, dtype=np.float32)
    W_attn = np.asarray(W_attn, dtype=np.float32)
    b_attn = np.asarray(b_attn, dtype=np.float32)
    W_proj = np.asarray(W_proj, dtype=np.float32)
    b_proj = np.asarray(b_proj, dtype=np.float32)
    bsz, t, c = x.shape
    key = (bsz, t, c)
    if key not in _NC_CACHE:
        _NC_CACHE[key] = _build(N_CORES, bsz, t, c)
    nc = _NC_CACHE[key]
    maps = _in_maps(x, W_attn, b_attn, W_proj, b_proj)
    res = run_bass_kernel_spmd(nc, maps, core_ids=list(range(N_CORES)),
                               trace=_trace)
    out = np.concatenate([res.results[i]["out"] for i in range(N_CORES)],
                         axis=0).reshape(bsz, t, c).astype(np.float32)
    if _trace:
        kernel.last_exec_time_ns = res.exec_time_ns
    return out


# revision 30
# speedup vs baseline: 1.0384x; 1.0384x over previous
"""Causal self-attention (B=4, T=2048, C=1024, H=16) on 8 TRN2 NeuronCores.

Strategy: tensor-parallel over heads. Each core computes QKV for its 2 heads
(columns of W_attn), runs causal flash-style attention for those heads over
all batches, then AllToAll collectives redistribute Y^T (head-dim-major) so
each core holds all 1024 head-dims for its 1/8 token slice. Each core then
computes its disjoint token slice of the output projection; the host
concatenates the 8 slices.

Layouts (per core):
  xT      [C, B*T]   bf16  x transposed (host-staged)
  wqk     [C, 256]   bf16  W_attn cols [qA|qB|kA|kB] for the core's 2 heads
  wv      [C, 128]   bf16  W_attn v-cols [vA|vB]
  bqk     [256, 1]   f32   matching biases
  bv      [128, 1]   f32
  wpj     [C, C]     bf16  W_proj (replicated)
  bpj     [1, C]     f32   b_proj (replicated)

Attention per (batch, head-pair): scores are computed transposed,
S^T[key, q] = K^T.T @ Q^T, both heads row-packed into the 128x128 PE array
(K=64 each) writing one 2-bank PSUM tile (one exp covers both heads).
exp on ScalarE (scale=1/8 fused, no max-subtraction: scores are O(1));
causal handled by skipping k-tiles above the diagonal, slicing the exp to
the valid q-range, and one [128,128] triangular mask multiply per head per
diagonal k-tile.  AV uses V' = [ones64 | V] as the stationary operand so
the softmax denominator lands replicated on PSUM rows 0-63 (y on rows
64-127); normalization is one fast approximate reciprocal plus a multiply.

Scheduling: attention runs in two passes (even q-chunks of every batch,
then odd q-chunks).  The first AllToAll covers the even-chunk halves and
overlaps the second pass; the first half of the projection is interleaved
into the last batch's second pass.  The next batch's QKV work is emitted
as a queue of small bursts injected between attention k-tiles to keep the
PE dense (HAM warm) through the ACT-bound attention phases.
"""

import numpy as np
import ml_dtypes

import concourse.bass as bass
import concourse.bacc as bacc
import concourse.mybir as mybir
import concourse.tile as tile
from concourse.tile import add_dep_helper
from concourse.bass_utils import run_bass_kernel_spmd

N_CORES = 8
B, T, C, H, D = 4, 2048, 1024, 16, 64

BF16 = mybir.dt.bfloat16
F32 = mybir.dt.float32


def _build(n_cores=N_CORES, b=B, t=T, c=C):
    """Build the SPMD graph (identical on every core)."""
    nt = b * t                 # total tokens
    s = nt // n_cores          # tokens per output slice
    hs = s // 2                # half-slice (per-A2A) width
    ch = min(512, t)           # attention q-chunk width
    n_ch = t // ch             # chunks per batch
    nkc = c // 128             # contraction tiles over C
    n_vt = t // 128            # V token-tiles per batch
    n_tt = s // 128            # proj token-tiles per core

    nc = bacc.Bacc("TRN2", target_bir_lowering=False, debug=False,
                   num_devices=n_cores)

    xT = nc.dram_tensor("xT", [c, nt], BF16, kind="ExternalInput")
    wqk = nc.dram_tensor("wqk", [c, 256], BF16, kind="ExternalInput")
    wv = nc.dram_tensor("wv", [c, 128], BF16, kind="ExternalInput")
    bqk = nc.dram_tensor("bqk", [256, 1], F32, kind="ExternalInput")
    bv = nc.dram_tensor("bv", [128, 1], F32, kind="ExternalInput")
    wpj = nc.dram_tensor("wpj", [c, c], BF16, kind="ExternalInput")
    bpj = nc.dram_tensor("bpj", [1, c], F32, kind="ExternalInput")
    out_ext = nc.dram_tensor("out", [s, c], F32, kind="ExternalOutput")

    with tile.TileContext(nc) as tc:
        with (
            tc.tile_pool(name="sb", bufs=1) as sb,
            tc.tile_pool(name="ps", bufs=1, space="PSUM") as ps,
            tc.tile_pool(name="dr", bufs=1, space="DRAM") as dr,
        ):
            y_loc = [dr.tile([c, hs], BF16, name=f"y_loc{h}",
                             tag=f"y_loc{h}") for h in range(2)]
            y_gth = [dr.tile([c, hs], BF16, name=f"y_gth{h}",
                             tag=f"y_gth{h}") for h in range(2)]

            # ---- constants (no DMA deps, so compute starts early) ----
            trimask = sb.tile([128, 128], BF16, name="trimask", tag="trimask")
            nc.gpsimd.memset(trimask[:], 0.0)
            nc.gpsimd.affine_select(   # trimask[r, q] = 1 iff q >= r
                out=trimask[:], in_=trimask[:],
                compare_op=mybir.AluOpType.is_gt, fill=1.0,
                base=0, pattern=[[-1, 128]], channel_multiplier=1,
            )
            ident = sb.tile([128, 128], BF16, name="ident", tag="ident")
            nc.gpsimd.memset(ident[:], 0.0)
            nc.gpsimd.affine_select(
                out=ident[:], in_=ident[:],
                compare_op=mybir.AluOpType.not_equal, fill=1.0,
                base=0, pattern=[[-1, 128]], channel_multiplier=1,
            )
            ones64 = sb.tile([65, 128], F32, name="ones64", tag="ones64")
            nc.vector.memset(ones64[64:65, :], 1.0)

            bq_sb = sb.tile([128, 1], F32, name="bq_sb", tag="bq_sb")
            bk_sb = sb.tile([128, 1], F32, name="bk_sb", tag="bk_sb")
            bv_sb = sb.tile([128, 1], F32, name="bv_sb", tag="bv_sb")
            nc.sync.dma_start(bq_sb[:], bqk.ap()[0:128, :])
            nc.sync.dma_start(bk_sb[:], bqk.ap()[128:256, :])

            # batch-0 x^T tiles first in the DMA queue (critical path), with
            # the small attention weights interleaved just behind xt0
            xt0 = []
            wqk_sb, wv_sb = [], []
            for kc in range(nkc):
                x_t = sb.tile([128, t], BF16, name=f"xt0_{kc}",
                              tag="xt", bufs=nkc + 4)
                eng = nc.sync if kc % 2 == 0 else nc.scalar
                eng.dma_start(x_t[:], xT.ap()[128 * kc:128 * (kc + 1), 0:t])
                xt0.append(x_t)
                wq_t = sb.tile([128, 256], BF16, name=f"wqk{kc}", tag="wqk",
                               bufs=nkc)
                nc.sync.dma_start(wq_t[:], wqk.ap()[128 * kc:128 * (kc + 1), :])
                wqk_sb.append(wq_t)
            nc.sync.dma_start(bv_sb[:], bv.ap())
            for kc in range(nkc):
                wv_t = sb.tile([128, 128], BF16, name=f"wv{kc}", tag="wv",
                               bufs=nkc)
                nc.sync.dma_start(wv_t[:], wv.ap()[128 * kc:128 * (kc + 1), :])
                wv_sb.append(wv_t)

            def qkv_work(bi, st, xt_pre=None):
                """Generator emitting batch bi's x^T load + Q^T/K^T/V' compute
                in small bursts (injected between attention k-tiles)."""
                if xt_pre is not None:
                    xt = xt_pre
                else:
                    xt = []
                    for kc in range(nkc):
                        x_t = sb.tile([128, t], BF16, name=f"xt{bi}_{kc}",
                                      tag="xt", bufs=nkc + 4)
                        nc.sync.dma_start(
                            x_t[:], xT.ap()[128 * kc:128 * (kc + 1),
                                            t * bi:t * (bi + 1)])
                        xt.append(x_t)
                        if kc % 4 == 3:
                            yield
                qt_sb = sb.tile([128, t], BF16, name=f"qt{bi}", tag="qt",
                                bufs=b)
                kt_sb = sb.tile([128, t], BF16, name=f"kt{bi}", tag="kt",
                                bufs=b)
                st["qt"], st["kt"] = qt_sb, kt_sb
                for half, (dst, bias) in enumerate(((qt_sb, bq_sb),
                                                    (kt_sb, bk_sb))):
                    for n0 in range(0, t, 512):
                        pqk = ps.tile([128, 512], F32,
                                      name=f"pqk{bi}_{half}_{n0}",
                                      tag="qkps", bufs=2)
                        for kc in range(nkc):
                            nc.tensor.matmul(
                                pqk[:],
                                wqk_sb[kc][:, 128 * half:128 * (half + 1)],
                                xt[kc][:, n0:n0 + 512],
                                start=(kc == 0), stop=(kc == nkc - 1))
                            if kc == nkc // 2 - 1:
                                yield
                        nc.vector.tensor_scalar_add(dst[:, n0:n0 + 512],
                                                    pqk[:], bias[:])
                        yield
                # V^T (D-major, per-partition bias) then PE-transpose into
                # token-major V' tiles [128, 256] = [ones|vA|ones|vB]
                vt_sb = sb.tile([128, t], BF16, name=f"vt{bi}", tag="vt",
                                bufs=2)
                for n0 in range(0, t, 512):
                    pvt = ps.tile([128, 512], F32, name=f"pvt{bi}_{n0}",
                                  tag="qkps", bufs=2)
                    for kc in range(nkc):
                        nc.tensor.matmul(pvt[:], wv_sb[kc][:],
                                         xt[kc][:, n0:n0 + 512],
                                         start=(kc == 0), stop=(kc == nkc - 1))
                        if kc == nkc // 2 - 1:
                            yield
                    nc.vector.tensor_scalar_add(vt_sb[:, n0:n0 + 512],
                                                pvt[:], bv_sb[:])
                    yield
                v_sb = st["v"] = []
                for tt in range(n_vt):
                    ptr = ps.tile([128, 128], BF16, name=f"ptr{bi}_{tt}",
                                  tag="qkps", bufs=2)
                    nc.tensor.transpose(ptr[:],
                                        vt_sb[:, 128 * tt:128 * (tt + 1)],
                                        ident[:])
                    v_t = sb.tile([128, 256], BF16, name=f"v{bi}_{tt}",
                                  tag="v", bufs=b * n_vt)
                    nc.vector.memset(v_t[:, 0:64], 1.0)
                    nc.vector.memset(v_t[:, 128:192], 1.0)
                    nc.vector.tensor_copy(v_t[:, 64:128], ptr[:, 0:64])
                    nc.vector.tensor_copy(v_t[:, 192:256], ptr[:, 64:128])
                    v_sb.append(v_t)
                    if tt % 2 == 1:
                        yield

            fence = [None]   # last attention-side instruction (ordering)

            def attn_chunk(bi, ci, st, inject, ys_eng=None):
                """Emit attention for (batch bi, q-chunk ci)."""
                if ys_eng is None:
                    ys_eng = nc.gpsimd
                qt_sb, kt_sb, v_sb = st["qt"], st["kt"], st["v"]
                q0 = ch * ci
                n_k = (q0 + ch) // 128
                ya = ps.tile([128, ch], F32, name=f"ya{bi}_{ci}",
                             tag="y", bufs=2)
                yb = ps.tile([128, ch], F32, name=f"yb{bi}_{ci}",
                             tag="y", bufs=2)
                # software pipeline: AV lags scores/exp by LAG k-tiles so
                # the PE never waits on ScalarE's exp (keeps PE dense/warm)
                LAG = 2
                p_tiles = {}
                for step in range(n_k + LAG):
                    if step < n_k:
                        kt = step
                        j0 = max(0, 128 * kt - q0)
                        # both heads' scores in one 2-bank psum tile:
                        # cols [0,ch) = head A, [ch,2ch) = head B
                        sAB = ps.tile([128, 2 * ch], F32,
                                      name=f"sAB{bi}_{ci}_{kt}",
                                      tag="sc", bufs=2)
                        nc.tensor.matmul(
                            sAB[:, j0:ch],
                            kt_sb[0:64, 128 * kt:128 * (kt + 1)],
                            qt_sb[0:64, q0 + j0:q0 + ch],
                            start=True, stop=True, tile_position=(0, 0))
                        nc.tensor.matmul(
                            sAB[:, ch + j0:2 * ch],
                            kt_sb[64:128, 128 * kt:128 * (kt + 1)],
                            qt_sb[64:128, q0 + j0:q0 + ch],
                            start=True, stop=True, tile_position=(64, 0))
                        pAB = sb.tile([128, 2 * ch], BF16,
                                      name=f"pAB{bi}_{ci}_{kt}",
                                      tag="p", bufs=6)
                        nc.scalar.activation(pAB[:, j0:2 * ch],
                                             sAB[:, j0:2 * ch],
                                             mybir.ActivationFunctionType.Exp,
                                             scale=0.125)
                        if 128 * kt >= q0:   # diagonal: triangular mask
                            nc.vector.tensor_mul(pAB[:, j0:j0 + 128],
                                                 pAB[:, j0:j0 + 128],
                                                 trimask[:])
                            nc.vector.tensor_mul(
                                pAB[:, ch + j0:ch + j0 + 128],
                                pAB[:, ch + j0:ch + j0 + 128], trimask[:])
                        p_tiles[kt] = pAB
                    if step >= LAG:
                        kt = step - LAG
                        j0 = max(0, 128 * kt - q0)
                        pAB = p_tiles.pop(kt)
                        nc.tensor.matmul(ya[:, j0:ch], v_sb[kt][:, 0:128],
                                         pAB[:, j0:ch],
                                         start=(kt == 0),
                                         stop=(kt == n_k - 1),
                                         skip_group_check=True)
                        nc.tensor.matmul(yb[:, j0:ch], v_sb[kt][:, 128:256],
                                         pAB[:, ch + j0:2 * ch],
                                         start=(kt == 0),
                                         stop=(kt == n_k - 1),
                                         skip_group_check=True)
                    inject()
                # normalize: den replicated on psum rows 0-63, y on 64-127
                rsA = sb.tile([64, ch], F32, name=f"rsA{bi}_{ci}",
                              tag="rs", bufs=2)
                rsB = sb.tile([64, ch], F32, name=f"rsB{bi}_{ci}",
                              tag="rs", bufs=2)
                nc.vector.reciprocal_approx_fast(rsA[:], ya[0:64, :])
                nc.vector.reciprocal_approx_fast(rsB[:], yb[0:64, :])
                ysA = sb.tile([64, ch], BF16, name=f"ysA{bi}_{ci}",
                              tag="ys", bufs=4)
                ysB = sb.tile([64, ch], BF16, name=f"ysB{bi}_{ci}",
                              tag="ys", bufs=4)
                nc.vector.tensor_mul(ysA[:], ya[64:128, :], rsA[:])
                nc.vector.tensor_mul(ysB[:], yb[64:128, :], rsB[:])
                # store into A2A-input halves: shard j rows, half h
                u0 = 0
                while u0 < ch:
                    g = bi * t + q0 + u0          # global token
                    j = g // s
                    col = g % s
                    h = 0 if col < hs else 1
                    col_h = col - h * hs
                    seg = min(ch - u0, hs - col_h)
                    ys_eng.dma_start(
                        y_loc[h][128 * j:128 * j + 64, col_h:col_h + seg],
                        ysA[:, u0:u0 + seg])
                    fence[0] = ys_eng.dma_start(
                        y_loc[h][128 * j + 64:128 * (j + 1), col_h:col_h + seg],
                        ysB[:, u0:u0 + seg])
                    u0 += seg
                inject()

            def after_attn(inst):
                if fence[0] is not None:
                    add_dep_helper(inst.ins, fence[0].ins, sync=False,
                                   reason="post-attn ordering")
                return inst

            def proj_work(h, yg_pre=None):
                """Emit the output projection for half h of my token slice."""
                if yg_pre is not None:
                    yg_sb = yg_pre
                else:
                    yg_sb = []
                    for kc in range(nkc):
                        yg_t = sb.tile([128, hs], BF16, name=f"yg{h}_{kc}",
                                       tag="yg", bufs=2 * nkc)
                        after_attn(nc.gpsimd.dma_start(
                            yg_t[:], y_gth[h][128 * kc:128 * (kc + 1), :]))
                        yg_sb.append(yg_t)
                for tt in range(n_tt // 2):
                    o_sb = sb.tile([128, c], F32, name=f"os{h}_{tt}",
                                   tag="os", bufs=2)
                    for n0 in range(0, c, 512):
                        po = ps.tile([128, 512], F32, name=f"po{h}_{tt}_{n0}",
                                     tag="qkps", bufs=2)
                        for kc in range(nkc):
                            after_attn(nc.tensor.matmul(
                                po[:],
                                yg_sb[kc][:, 128 * tt:128 * (tt + 1)],
                                wpj_sb[kc][:, n0:n0 + 512],
                                start=(kc == 0), stop=(kc == nkc - 1)))
                        nc.vector.tensor_add(o_sb[:, n0:n0 + 512], po[:],
                                             bpj_bc[:, n0:n0 + 512])
                    nc.gpsimd.dma_start(
                        out_ext.ap()[hs * h + 128 * tt:
                                     hs * h + 128 * (tt + 1), :], o_sb[:])

            # ---- batch 0 QKV inline ----
            states = [{}]
            for _ in qkv_work(0, states[0], xt_pre=xt0):
                pass

            # proj weights + bias broadcast (DMA overlaps batch-0 attention)
            wpj_sb = []
            for kc in range(nkc):
                wp_t = sb.tile([128, c], BF16, name=f"wpj{kc}", tag="wpj",
                               bufs=nkc)
                nc.sync.dma_start(wp_t[:], wpj.ap()[128 * kc:128 * (kc + 1), :])
                wpj_sb.append(wp_t)
            brow = sb.tile([65, c], F32, name="brow", tag="brow")
            nc.sync.dma_start(brow[64:65, 0:c], bpj.ap())
            bpj_bc = sb.tile([128, c], F32, name="bpj_bc", tag="bpj_bc")
            for n0 in range(0, c, 512):
                bb_ps = ps.tile([128, 512], F32, name=f"bb_ps{n0}",
                                tag="qkps", bufs=2)
                nc.tensor.matmul(bb_ps[:],
                                 ones64[64:65, 0:128], brow[64:65, n0:n0 + 512],
                                 start=True, stop=True)
                nc.vector.tensor_copy(bpj_bc[:, n0:n0 + 512], bb_ps[:])

            # ---- pass 1: even chunks, next batch's QKV interleaved ----
            if n_ch >= 2:
                passes = ([ci for ci in range(n_ch) if ci % 2 == 0],
                          [ci for ci in range(n_ch) if ci % 2 == 1])
            else:
                passes = ([0], [])
            pending = iter(())
            for bi in range(b):
                for _ in pending:
                    pass
                if bi + 1 < b:
                    states.append({})
                    pending = qkv_work(bi + 1, states[bi + 1])
                else:
                    pending = iter(())

                def inject2():
                    next(pending, None)
                    next(pending, None)
                for ci in passes[0]:
                    attn_chunk(bi, ci, states[bi], inject2)
            for _ in pending:
                pass

            # A2A #1: even-chunk halves (overlaps pass-2 compute)
            nc.gpsimd.collective_compute(
                "AllToAll", mybir.AluOpType.bypass,
                replica_groups=[list(range(n_cores))],
                ins=[y_loc[0].opt()], outs=[y_gth[0].opt()],
            )

            # ---- pass 2: odd chunks ----
            def noop():
                pass
            for bi in range(b):
                for ci in passes[1]:
                    attn_chunk(bi, ci, states[bi], noop)

            # gathered half-0 loads, A2A #2, then both projection halves
            nc.gpsimd.collective_compute(
                "AllToAll", mybir.AluOpType.bypass,
                replica_groups=[list(range(n_cores))],
                ins=[y_loc[1].opt()], outs=[y_gth[1].opt()],
            )
            yg0 = []
            for kc in range(nkc):
                yg_t = sb.tile([128, hs], BF16, name=f"yg0_{kc}",
                               tag="yg", bufs=2 * nkc)
                after_attn(nc.gpsimd.dma_start(
                    yg_t[:], y_gth[0][128 * kc:128 * (kc + 1), :]))
                yg0.append(yg_t)
            proj_work(0, yg0)
            proj_work(1)

    nc.compile()
    return nc


def _in_maps(x, W_attn, b_attn, W_proj, b_proj, n_cores=N_CORES):
    bsz, t, c = x.shape
    xT = np.ascontiguousarray(
        x.reshape(bsz * t, c).T).astype(ml_dtypes.bfloat16)
    wpj = W_proj.astype(ml_dtypes.bfloat16)
    bpj = b_proj.reshape(1, c).astype(np.float32)
    maps = []
    hpc = 2
    d = c // 16
    for i in range(n_cores):
        cols, bcols, vcols, bvcols = [], [], [], []
        for h in (hpc * i, hpc * i + 1):
            cols.append(W_attn[:, d * h:d * (h + 1)])          # q
            bcols.append(b_attn[d * h:d * (h + 1)])
        for h in (hpc * i, hpc * i + 1):
            cols.append(W_attn[:, c + d * h:c + d * (h + 1)])  # k
            bcols.append(b_attn[c + d * h:c + d * (h + 1)])
        for h in (hpc * i, hpc * i + 1):
            vcols.append(W_attn[:, 2 * c + d * h:2 * c + d * (h + 1)])
            bvcols.append(b_attn[2 * c + d * h:2 * c + d * (h + 1)])
        maps.append({
            "xT": xT,
            "wqk": np.concatenate(cols, axis=1).astype(ml_dtypes.bfloat16),
            "wv": np.concatenate(vcols, axis=1).astype(ml_dtypes.bfloat16),
            "bqk": np.concatenate(bcols).reshape(256, 1).astype(np.float32),
            "bv": np.concatenate(bvcols).reshape(128, 1).astype(np.float32),
            "wpj": wpj,
            "bpj": bpj,
        })
    return maps


_NC_CACHE = {}


def kernel(x, W_attn, b_attn, W_proj, b_proj, _trace=False):
    x = np.asarray(x, dtype=np.float32)
    W_attn = np.asarray(W_attn, dtype=np.float32)
    b_attn = np.asarray(b_attn, dtype=np.float32)
    W_proj = np.asarray(W_proj, dtype=np.float32)
    b_proj = np.asarray(b_proj, dtype=np.float32)
    bsz, t, c = x.shape
    key = (bsz, t, c)
    if key not in _NC_CACHE:
        _NC_CACHE[key] = _build(N_CORES, bsz, t, c)
    nc = _NC_CACHE[key]
    maps = _in_maps(x, W_attn, b_attn, W_proj, b_proj)
    res = run_bass_kernel_spmd(nc, maps, core_ids=list(range(N_CORES)),
                               trace=_trace)
    out = np.concatenate([res.results[i]["out"] for i in range(N_CORES)],
                         axis=0).reshape(bsz, t, c).astype(np.float32)
    if _trace:
        kernel.last_exec_time_ns = res.exec_time_ns
    return out
